# revision 7
# baseline (speedup 1.0000x reference)
"""GPT-2-small forward pass on 8 Trainium2 NeuronCores.

Sharding: 2 data-parallel groups of 4 cores (one per batch element).  Within a
group each core owns 256 tokens (query tiles {c, 7-c} of the 8x128-token tiles,
mirrored pairing so every core has exactly 9 causal k-blocks of attention work).
Per layer the only communication is one 4-rank AllGather of the local K^T/V
slabs.  FFN / LN / residual are fully local with replicated bf16 weights.
The lm_head is vocab-sharded 8 ways after a final 8-rank AllGather of the
hidden states; the host concatenates the logit shards.

Attention is a fully static 12-slot schedule per head (uniform SPMD program):
slots 0/1 are the two diagonal blocks (read from the core's *local* K/V copy at
static offsets, triangular mask added in PSUM), slots 2-4 are off-diagonal
k-blocks 0-2 for the low query tile, slots 5-11 are k-blocks 0-6 for the high
tile.  Unused slots are disabled via a per-core [128,12] bias table that feeds
the exp's ACT bias (-1e9 -> exp==0), so per-core causal asymmetry is pure data.
"""

import sys

sys.path.insert(0, "/opt/trn_rl_repo")

import numpy as np
import ml_dtypes

import concourse.bass as bass
import concourse.bacc as bacc
import concourse.mybir as mybir
import concourse.tile as tile
import concourse.tile_utils as tile_utils
from concourse.bass_utils import run_bass_kernel_spmd

BF16_NP = ml_dtypes.bfloat16
F32 = mybir.dt.float32
BF16 = mybir.dt.bfloat16
AF = mybir.ActivationFunctionType

V, H, L, NH, I, PMAX = 50257, 768, 12, 12, 3072, 2048
B, S = 2, 1024
HD = H // NH          # 64
T = 256               # local tokens per core
NC_ = 8               # cores
VS = 6283             # vocab shard per core (8*6283 = 50264 >= 50257, zero-padded)
VPAD = NC_ * VS
NEG = -1e9

# lift tile's stale SBUF cap (cayman has 208KB/partition usable)
tile_utils.max_sbuf_usage = 204 * 1024

_prog_cache = {}
last_exec_time_ns = None


def _build_program(n_layers):
    nc = bacc.Bacc("TRN2", target_bir_lowering=False, debug=False, num_devices=NC_)

    # ---- DRAM I/O ----
    x0t_d = nc.dram_tensor("x0t", [H, T], F32, kind="ExternalInput")
    tri_d = nc.dram_tensor("tri", [128, 128], F32, kind="ExternalInput")
    mtab_d = nc.dram_tensor("mtab", [128, 12], F32, kind="ExternalInput")
    wq_d = nc.dram_tensor("wqT", [n_layers, H, H], BF16, kind="ExternalInput")
    wk_d = nc.dram_tensor("wkT", [n_layers, H, H], BF16, kind="ExternalInput")
    wv_d = nc.dram_tensor("wvT", [n_layers, H, H], BF16, kind="ExternalInput")
    wo_d = nc.dram_tensor("woT", [n_layers, H, H], BF16, kind="ExternalInput")
    w1_d = nc.dram_tensor("w1T", [n_layers, H, I], BF16, kind="ExternalInput")
    w2_d = nc.dram_tensor("w2T", [n_layers, I, H], BF16, kind="ExternalInput")
    lm_d = nc.dram_tensor("lmT", [H, VS], BF16, kind="ExternalInput")
    out_d = nc.dram_tensor("logits_sh", [B * S, VS], F32, kind="ExternalOutput")

    KS = H * T            # 196608 elems in a K^T / V slab
    KVS = 2 * KS
    kv_in = [nc.dram_tensor(f"kvin{l}", [KVS], BF16) for l in range(n_layers)]
    kv_out = [
        nc.dram_tensor(f"kvout{l}", [4 * KVS], BF16) for l in range(n_layers)
    ]
    hf_in = nc.dram_tensor("hfin", [KS], BF16)
    hf_out = nc.dram_tensor("hfout", [NC_ * KS], BF16, addr_space="Shared")

    KV_GROUPS = [[0, 1, 2, 3], [4, 5, 6, 7]]
    ALL_GROUPS = [list(range(NC_))]

    with tile.TileContext(nc) as tc:
        _trace(tc, n_layers, x0t_d, tri_d, mtab_d, wq_d, wk_d, wv_d, wo_d,
               w1_d, w2_d, lm_d, out_d, kv_in, kv_out, hf_in, hf_out,
               KV_GROUPS, ALL_GROUPS)
    nc.compile()
    return nc


def _trace(tc, n_layers, x0t_d, tri_d, mtab_d, wq_d, wk_d, wv_d, wo_d,
           w1_d, w2_d, lm_d, out_d, kv_in, kv_out, hf_in, hf_out,
           KV_GROUPS, ALL_GROUPS):
    nc = tc.nc
    KS = H * T

    with (
        tc.tile_pool(name="pers", bufs=1) as pers,
        tc.tile_pool(name="psum_g", bufs=2, space="PSUM") as psum_g,
        tc.tile_pool(name="psum_sc", bufs=3, space="PSUM") as psum_sc,
        tc.tile_pool(name="psum_av", bufs=2, space="PSUM") as psum_av,
        tc.tile_pool(name="psum_pb", bufs=1, space="PSUM") as psum_pb,
        tc.tile_pool(name="sm", bufs=2) as sm,
        tc.tile_pool(name="ptp", bufs=8) as ptp,
    ):
        # persistent SBUF state
        x_sb = pers.tile([128, 6 * T], F32, tag="x")          # residual, feat-major
        xb = pers.tile([128, 6 * T], BF16, tag="xb")          # bf16 copy for LN sums
        hln = pers.tile([128, 6 * T], BF16, tag="hln")        # LN out (+ x^2 scratch)
        q_sb = pers.tile([128, 6 * T], BF16, tag="q")         # Q^T local
        kloc = pers.tile([128, 6 * T], BF16, tag="kloc")      # K^T local
        vloc = pers.tile([128, 2 * 780], BF16, tag="vloc")    # V local, 65-interleaved
        ksb = pers.tile([128, 6 * 896], BF16, tag="ksb")      # K^T gathered, blocks 0-6
        vsb = pers.tile([128, 7 * 780], BF16, tag="vsb")      # V gathered, blocks 0-6
        a_sb = pers.tile([128, 6 * T], BF16, tag="a")         # attn out (a^T)
        g_sb = pers.tile([128, 24 * T], BF16, tag="g")        # gelu(FFN1) out
        tri_sb = pers.tile([128, 128], F32, tag="tri")
        mtab = pers.tile([128, 12], F32, tag="mtab")
        ones_k = pers.tile([128, 1], BF16, tag="ok")          # lhsT for col-sums
        ones_b = pers.tile([1, 128], BF16, tag="ob")          # lhsT for broadcasts

        nc.sync.dma_start(tri_sb[:, :], tri_d[:, :])
        nc.sync.dma_start(mtab[:, :], mtab_d[:, :])
        nc.vector.memset(ones_k[:, :], 1.0)
        nc.vector.memset(ones_b[:, :], 1.0)
        # ones columns (col 64 of each 65-wide head slot) for the softmax denom
        nc.vector.memset(
            vloc[:, :].rearrange("p (t h c) -> p t h c", t=2, h=12)[:, :, :, 64:65], 1.0
        )
        nc.vector.memset(
            vsb[:, :].rearrange("p (t h c) -> p t h c", t=7, h=12)[:, :, :, 64:65], 1.0
        )
        for j in range(6):
            nc.sync.dma_start(x_sb[:, T * j : T * (j + 1)], x0t_d[128 * j : 128 * (j + 1), :])

        def layer_norm(wtag):
            """x_sb -> hln (bf16). ln scale/bias pre-folded into weights host-side."""
            nc.scalar.copy(xb[:, :], x_sb[:, :])
            nc.scalar.activation(hln[:, :], xb[:, :], AF.Square)
            ps_s = psum_g.tile([1, T], F32, tag="g")
            ps_q = psum_g.tile([1, T], F32, tag="g")
            for k in range(6):
                nc.tensor.matmul(ps_s[:, :], ones_k[:, :], xb[:, T * k : T * (k + 1)],
                                 start=(k == 0), stop=(k == 5))
            for k in range(6):
                nc.tensor.matmul(ps_q[:, :], ones_k[:, :], hln[:, T * k : T * (k + 1)],
                                 start=(k == 0), stop=(k == 5))
            mean = sm.tile([1, T], F32, tag="mean")
            var = sm.tile([1, T], F32, tag="var")
            rstd = sm.tile([1, T], F32, tag="rstd")
            b0 = sm.tile([1, T], F32, tag="b0")
            rb = sm.tile([1, 2 * T], BF16, tag="rb")
            nc.vector.tensor_scalar_mul(mean[:, :], ps_s[:, :], 1.0 / H)
            nc.vector.tensor_scalar_mul(var[:, :], ps_q[:, :], 1.0 / H)
            nc.vector.tensor_tensor(b0[:, :], mean[:, :], mean[:, :], mybir.AluOpType.mult)
            nc.vector.tensor_sub(var[:, :], var[:, :], b0[:, :])
            nc.vector.tensor_scalar_add(var[:, :], var[:, :], 1e-5)
            nc.scalar.activation(var[:, :], var[:, :], AF.Sqrt)
            nc.vector.reciprocal(rstd[:, :], var[:, :])
            nc.vector.tensor_tensor(b0[:, :], mean[:, :], rstd[:, :], mybir.AluOpType.mult)
            nc.vector.tensor_scalar_mul(b0[:, :], b0[:, :], -1.0)
            nc.scalar.copy(rb[:, 0:T], rstd[:, :])
            nc.scalar.copy(rb[:, T : 2 * T], b0[:, :])
            psb = psum_g.tile([128, 2 * T], F32, tag="g")
            nc.tensor.matmul(psb[:, :], ones_b[:, :], rb[:, :], start=True, stop=True)
            for k in range(6):
                sl = slice(T * k, T * (k + 1))
                nc.vector.tensor_tensor(hln[:, sl], x_sb[:, sl], psb[:, 0:T],
                                        mybir.AluOpType.mult)
                nc.vector.tensor_tensor(hln[:, sl], hln[:, sl], psb[:, T : 2 * T],
                                        mybir.AluOpType.add)

        def gemm_fm(w_t, dout, dst, dst_bf16=True, act=None, add_to_x=False):
            """out^T[dout, T] = W @ act_in^T ; lhsT slabs in w_t [128, 6*dout]."""
            rhs = act if act is not None else hln
            for m in range(dout // 128):
                ps = psum_g.tile([128, T], F32, tag="g")
                nk = w_t.shape[1] // dout
                for k in range(nk):
                    nc.tensor.matmul(
                        ps[:, :],
                        w_t[:, dout * k + 128 * m : dout * k + 128 * (m + 1)],
                        rhs[:, T * k : T * (k + 1)],
                        start=(k == 0), stop=(k == nk - 1),
                    )
                sl = slice(T * m, T * (m + 1))
                if add_to_x:
                    nc.vector.tensor_tensor(x_sb[:, sl], x_sb[:, sl], ps[:, :],
                                            mybir.AluOpType.add)
                elif act is not None and dst_bf16 == "gelu":
                    nc.scalar.activation(dst[:, sl], ps[:, :], AF.Gelu)
                else:
                    nc.scalar.copy(dst[:, sl], ps[:, :])

        for l in range(n_layers):
            with (
                tc.tile_pool(name=f"wqp{l % 2}", bufs=1) as wqp,
                tc.tile_pool(name=f"wkp{l % 2}", bufs=1) as wkp,
                tc.tile_pool(name=f"wvp{l % 2}", bufs=1) as wvp,
                tc.tile_pool(name=f"wop{l % 2}", bufs=1) as wop,
                tc.tile_pool(name=f"w1p{l % 2}", bufs=1) as w1p,
                tc.tile_pool(name=f"w2p{l % 2}", bufs=1) as w2p,
            ):
                wq_t = wqp.tile([128, 6 * H], BF16)
                wk_t = wkp.tile([128, 6 * H], BF16)
                wv_t = wvp.tile([128, 6 * H], BF16)
                wo_t = wop.tile([128, 6 * H], BF16)
                w1_t = w1p.tile([128, 6 * I], BF16)
                w2_t = w2p.tile([128, 24 * H], BF16)
                for k in range(6):
                    r = slice(128 * k, 128 * (k + 1))
                    nc.sync.dma_start(wq_t[:, H * k : H * (k + 1)], wq_d[l, r, :])
                    nc.sync.dma_start(wk_t[:, H * k : H * (k + 1)], wk_d[l, r, :])
                    nc.sync.dma_start(wv_t[:, H * k : H * (k + 1)], wv_d[l, r, :])
                    nc.sync.dma_start(wo_t[:, H * k : H * (k + 1)], wo_d[l, r, :])
                    nc.sync.dma_start(w1_t[:, I * k : I * (k + 1)], w1_d[l, r, :])
                for k in range(24):
                    nc.sync.dma_start(w2_t[:, H * k : H * (k + 1)],
                                      w2_d[l, 128 * k : 128 * (k + 1), :])

                # ---- LN1 + QKV ----
                layer_norm("1")
                gemm_fm(wk_t, H, kloc)
                # V token-major: V[tok, feat] = hln^T chunks as lhsT, wv as rhs
                for tt in range(2):
                    for half in range(2):
                        ps = psum_g.tile([128, 384], F32, tag="g")
                        for k in range(6):
                            nc.tensor.matmul(
                                ps[:, :],
                                hln[:, T * k + 128 * tt : T * k + 128 * (tt + 1)],
                                wv_t[:, H * k + 384 * half : H * k + 384 * (half + 1)],
                                start=(k == 0), stop=(k == 5),
                            )
                        dst = vloc[:, 780 * tt : 780 * (tt + 1)].rearrange(
                            "p (h c) -> p h c", c=65)[:, 6 * half : 6 * (half + 1), 0:64]
                        nc.scalar.copy(dst, ps[:, :].rearrange("p (h c) -> p h c", c=64))
                gemm_fm(wq_t, H, q_sb)

                # ---- ship K/V into the AG bounce, run AllGather ----
                kv_k = kv_in[l].ap()[0:KS].rearrange("(p f) -> p f", p=H)
                kv_v = kv_in[l].ap()[KS : 2 * KS].rearrange("(p f) -> p f", p=T)
                for m in range(6):
                    nc.sync.dma_start(kv_k[128 * m : 128 * (m + 1), :],
                                      kloc[:, T * m : T * (m + 1)])
                for tt in range(2):
                    src = vloc[:, 780 * tt : 780 * (tt + 1)].rearrange(
                        "p (h c) -> p h c", c=65)[:, :, 0:64]
                    nc.sync.dma_start(
                        kv_v[128 * tt : 128 * (tt + 1), :].rearrange(
                            "p (h c) -> p h c", c=64), src)
                nc.gpsimd.collective_compute(
                    "AllGather", mybir.AluOpType.bypass, replica_groups=KV_GROUPS,
                    ins=[kv_in[l].ap().opt()], outs=[kv_out[l].ap().opt()])

                # ---- load gathered K/V (global k-blocks 0..6) ----
                for r in range(4):
                    slab_k = kv_out[l].ap()[2 * KS * r : 2 * KS * r + KS].rearrange(
                        "(p f) -> p f", p=H)
                    slab_v = kv_out[l].ap()[2 * KS * r + KS : 2 * KS * (r + 1)].rearrange(
                        "(p f) -> p f", p=T)
                    for j in range(6):
                        src = slab_k[128 * j : 128 * (j + 1), :]
                        for half, t in ((0, r), (1, 7 - r)):
                            if t == 7:
                                continue
                            nc.sync.dma_start(
                                ksb[:, 896 * j + 128 * t : 896 * j + 128 * (t + 1)],
                                src[:, 128 * half : 128 * (half + 1)])
                    for half in range(2):
                        t = r if half == 0 else 7 - r
                        if t == 7:
                            continue
                        dst = vsb[:, 780 * t : 780 * (t + 1)].rearrange(
                            "p (h c) -> p h c", c=65)[:, :, 0:64]
                        nc.sync.dma_start(
                            dst, slab_v[128 * half : 128 * (half + 1), :].rearrange(
                                "p (h c) -> p h c", c=64))

                # ---- attention: 12 static slots per head ----
                for h in range(NH):
                    jh, po = h // 2, (h % 2) * 64
                    avL = psum_av.tile([65, 128], F32, tag="av")
                    avH = psum_av.tile([65, 128], F32, tag="av")
                    # (slot, is_diag, qt, av_psum, av_start, av_stop, k_lhsT, v_lhsT)
                    slots = []
                    for s in range(12):
                        if s < 2:
                            qt = s
                            klh = kloc[po : po + 64, T * jh + 128 * qt : T * jh + 128 * (qt + 1)]
                            vlh = vloc[:, 780 * qt + 65 * h : 780 * qt + 65 * (h + 1)]
                            diag = True
                        elif s < 5:
                            qt, j = 0, s - 2
                            klh = ksb[po : po + 64, 896 * jh + 128 * j : 896 * jh + 128 * (j + 1)]
                            vlh = vsb[:, 780 * j + 65 * h : 780 * j + 65 * (h + 1)]
                            diag = False
                        else:
                            qt, j = 1, s - 5
                            klh = ksb[po : po + 64, 896 * jh + 128 * j : 896 * jh + 128 * (j + 1)]
                            vlh = vsb[:, 780 * j + 65 * h : 780 * j + 65 * (h + 1)]
                            diag = False
                        slots.append((s, diag, qt, klh, vlh))
                    last_for = {0: 4, 1: 11}
                    for s, diag, qt, klh, vlh in slots:
                        qs = q_sb[po : po + 64, T * jh + 128 * qt : T * jh + 128 * (qt + 1)]
                        ps = psum_sc.tile([128, 128], F32, tag="sc")
                        nc.tensor.matmul(ps[:, :], klh, qs, start=True, stop=True)
                        if diag:
                            nc.vector.tensor_tensor(ps[:, :], ps[:, :], tri_sb[:, :],
                                                    mybir.AluOpType.add)
                        p_t = ptp.tile([128, 128], BF16, tag="pt")
                        nc.scalar.activation(p_t[:, :], ps[:, :], AF.Exp,
                                             bias=mtab[:, s : s + 1])
                        av = avL if qt == 0 else avH
                        nc.tensor.matmul(av[:, :], vlh, p_t[:, :],
                                         start=(s == qt), stop=(s == last_for[qt]))
                    # softmax denominators -> divide -> a_sb
                    recip = sm.tile([1, T], F32, tag="recip")
                    nc.vector.reciprocal(recip[:, 0:128], avL[64:65, :])
                    nc.vector.reciprocal(recip[:, 128:256], avH[64:65, :])
                    rq = sm.tile([1, T], BF16, tag="rq")
                    nc.scalar.copy(rq[:, :], recip[:, :])
                    pb = psum_pb.tile([64, T], F32, tag="pb")
                    nc.tensor.matmul(pb[:, :], ones_b[:, 0:64], rq[:, :],
                                     start=True, stop=True)
                    rb2 = sm.tile([64, T], BF16, tag="rb2")
                    nc.scalar.copy(rb2[:, :], pb[:, :])
                    asl = a_sb[po : po + 64, T * jh : T * (jh + 1)]
                    nc.vector.tensor_tensor(asl[:, 0:128], avL[0:64, :], rb2[:, 0:128],
                                            mybir.AluOpType.mult)
                    nc.vector.tensor_tensor(asl[:, 128:256], avH[0:64, :], rb2[:, 128:256],
                                            mybir.AluOpType.mult)

                # ---- O proj (+ residual), LN2, FFN (+ residual) ----
                gemm_fm(wo_t, H, None, act=a_sb, add_to_x=True)
                layer_norm("2")
                gemm_fm(w1_t, I, g_sb, act=hln, dst_bf16="gelu")
                gemm_fm(w2_t, H, None, act=g_sb, add_to_x=True)

        # ---- final LN, AllGather h, vocab-sharded lm_head ----
        layer_norm("f")
        for m in range(6):
            nc.sync.dma_start(
                hf_in.ap().rearrange("(p f) -> p f", p=H)[128 * m : 128 * (m + 1), :],
                hln[:, T * m : T * (m + 1)])
        nc.gpsimd.collective_compute(
            "AllGather", mybir.AluOpType.bypass, replica_groups=ALL_GROUPS,
            ins=[hf_in.ap().opt()], outs=[hf_out.ap().opt()])

        with (
            tc.tile_pool(name="hsb", bufs=1) as hsbp,
            tc.tile_pool(name="lmw", bufs=12) as lmwp,
            tc.tile_pool(name="lo", bufs=4) as lop,
        ):
            h_sb = hsbp.tile([128, 6 * 2048], BF16)
            for r in range(NC_):
                g, c = r // 4, r % 4
                slab = hf_out.ap()[KS * r : KS * (r + 1)].rearrange("(p f) -> p f", p=H)
                for j in range(6):
                    for half, gt in ((0, c), (1, 7 - c)):
                        col = 1024 * g + 128 * gt
                        nc.sync.dma_start(
                            h_sb[:, 2048 * j + col : 2048 * j + col + 128],
                            slab[128 * j : 128 * (j + 1), 128 * half : 128 * (half + 1)])
            nv_sizes = [512] * 12 + [VS - 12 * 512]
            for v in range(13):
                nv = nv_sizes[v]
                lw = []
                for k in range(6):
                    t = lmwp.tile([128, 512], BF16, tag="lw")
                    nc.sync.dma_start(t[:, 0:nv], lm_d[128 * k : 128 * (k + 1),
                                                       512 * v : 512 * v + nv])
                    lw.append(t)
                for m in range(16):
                    ps = psum_g.tile([128, 512], F32, tag="g")
                    for k in range(6):
                        nc.tensor.matmul(
                            ps[:, 0:nv],
                            h_sb[:, 2048 * k + 128 * m : 2048 * k + 128 * (m + 1)],
                            lw[k][:, 0:nv], start=(k == 0), stop=(k == 5))
                    lo = lop.tile([128, 512], F32, tag="lo")
                    nc.scalar.copy(lo[:, 0:nv], ps[:, 0:nv])
                    nc.sync.dma_start(out_d[128 * m : 128 * (m + 1), 512 * v : 512 * v + nv],
                                      lo[:, 0:nv])


def kernel(input_ids, attention_mask, tok_emb, pos_emb, ln1_w, ln1_b, qw, kw, vw,
           ow, ob, ln2_w, ln2_b, w1, b1, w2, b2, lnf_w, lnf_b, lm_head_w):
    n_layers = int(qw.shape[0])
    for b in (ln1_b, ob, ln2_b, b1, b2, lnf_b):
        assert np.abs(np.asarray(b)).max() == 0.0, "nonzero biases unsupported"

    input_ids = np.asarray(input_ids)
    x0 = np.asarray(tok_emb)[input_ids] + np.asarray(pos_emb)[:S][None]  # [B,S,H] f32

    scale = 1.0 / np.sqrt(HD)
    ln1_w = np.asarray(ln1_w, np.float32)
    ln2_w = np.asarray(ln2_w, np.float32)
    wqT = np.ascontiguousarray(
        np.transpose(np.asarray(qw), (0, 2, 1)) * ln1_w[:, :, None] * scale).astype(BF16_NP)
    wkT = np.ascontiguousarray(
        np.transpose(np.asarray(kw), (0, 2, 1)) * ln1_w[:, :, None]).astype(BF16_NP)
    wvT = np.ascontiguousarray(
        np.transpose(np.asarray(vw), (0, 2, 1)) * ln1_w[:, :, None]).astype(BF16_NP)
    woT = np.ascontiguousarray(np.transpose(np.asarray(ow), (0, 2, 1))).astype(BF16_NP)
    w1T = np.ascontiguousarray(
        np.transpose(np.asarray(w1), (0, 2, 1)) * ln2_w[:, :, None]).astype(BF16_NP)
    w2T = np.ascontiguousarray(np.transpose(np.asarray(w2), (0, 2, 1))).astype(BF16_NP)
    lm_pad = np.zeros((VPAD, H), np.float32)
    lm_pad[:V] = np.asarray(lm_head_w) * np.asarray(lnf_w, np.float32)[None, :]

    tri = np.where(np.arange(128)[:, None] <= np.arange(128)[None, :], 0.0, NEG
                   ).astype(np.float32)
    am = np.asarray(attention_mask)
    kbias = np.where(am != 0, 0.0, NEG).astype(np.float32)  # [B,S] key-pad bias

    in_maps = []
    for cid in range(NC_):
        g, c = cid // 4, cid % 4
        tiles = (c, 7 - c)
        x0t = np.ascontiguousarray(np.concatenate(
            [x0[g, 128 * t : 128 * (t + 1)] for t in tiles], axis=0).T.astype(np.float32))
        mtab = np.zeros((128, 12), np.float32)
        mtab[:, 0] = kbias[g, 128 * c : 128 * (c + 1)]
        mtab[:, 1] = kbias[g, 128 * (7 - c) : 128 * (8 - c)]
        for j in range(3):
            mtab[:, 2 + j] = (kbias[g, 128 * j : 128 * (j + 1)] if j < c else NEG)
        for j in range(7):
            mtab[:, 5 + j] = (kbias[g, 128 * j : 128 * (j + 1)] if j <= 6 - c else NEG)
        lmT = np.ascontiguousarray(lm_pad[VS * cid : VS * (cid + 1)].T).astype(BF16_NP)
        in_maps.append({
            "x0t": x0t, "tri": tri, "mtab": mtab,
            "wqT": wqT, "wkT": wkT, "wvT": wvT, "woT": woT, "w1T": w1T, "w2T": w2T,
            "lmT": lmT,
        })

    if n_layers not in _prog_cache:
        _prog_cache[n_layers] = _build_program(n_layers)
    nc = _prog_cache[n_layers]
    import os
    _trace = bool(os.environ.get("KTRACE"))
    try:
        res = run_bass_kernel_spmd(nc, in_maps, core_ids=list(range(NC_)), trace=_trace)
    except ModuleNotFoundError:
        res = run_bass_kernel_spmd(nc, in_maps, core_ids=list(range(NC_)))
    global last_exec_time_ns
    last_exec_time_ns = res.exec_time_ns
    full = np.concatenate([res.results[c]["logits_sh"] for c in range(NC_)], axis=1)
    return np.ascontiguousarray(full[:, :V].reshape(B, S, V).astype(np.float32))


# revision 8
# speedup vs baseline: 1.1891x; 1.1891x over previous
"""GPT-2-small forward pass on 8 Trainium2 NeuronCores.

Sharding: 2 data-parallel groups of 4 cores (one per batch element).  Within a
group each core owns 256 tokens (query tiles {c, 7-c} of the 8x128-token tiles,
mirrored pairing so every core has exactly 9 causal k-blocks of attention work).
Per layer the only communication is one 4-rank AllGather of the local K^T/V
slabs.  FFN / LN / residual are fully local with replicated bf16 weights.
The lm_head is vocab-sharded 8 ways after a final 8-rank AllGather of the
hidden states; the host concatenates the logit shards.

Attention is a fully static 12-slot schedule per head (uniform SPMD program):
slots 0/1 are the two diagonal blocks (read from the core's *local* K/V copy at
static offsets, triangular mask added in PSUM), slots 2-4 are off-diagonal
k-blocks 0-2 for the low query tile, slots 5-11 are k-blocks 0-6 for the high
tile.  Unused slots are disabled via a per-core [128,12] bias table that feeds
the exp's ACT bias (-1e9 -> exp==0), so per-core causal asymmetry is pure data.
"""

import os
import sys

sys.path.insert(0, "/opt/trn_rl_repo")
os.environ.setdefault("NEURON_RT_EXEC_TIMEOUT", "600")

import numpy as np
import ml_dtypes

import concourse.bass as bass
import concourse.bacc as bacc
import concourse.mybir as mybir
import concourse.tile as tile
import concourse.tile_utils as tile_utils
from concourse.bass_utils import run_bass_kernel_spmd

BF16_NP = ml_dtypes.bfloat16
F32 = mybir.dt.float32
BF16 = mybir.dt.bfloat16
AF = mybir.ActivationFunctionType

V, H, L, NH, I, PMAX = 50257, 768, 12, 12, 3072, 2048
B, S = 2, 1024
HD = H // NH          # 64
T = 256               # local tokens per core
NC_ = 8               # cores
VS = 6283             # vocab shard per core (8*6283 = 50264 >= 50257, zero-padded)
VPAD = NC_ * VS
NEG = -1e9

# lift tile's stale SBUF cap (cayman has 208KB/partition usable)
tile_utils.max_sbuf_usage = 204 * 1024

_prog_cache = {}
last_exec_time_ns = None


def _build_program(n_layers):
    nc = bacc.Bacc("TRN2", target_bir_lowering=False, debug=False, num_devices=NC_)

    # ---- DRAM I/O ----
    x0t_d = nc.dram_tensor("x0t", [H, T], F32, kind="ExternalInput")
    tri_d = nc.dram_tensor("tri", [128, 128], F32, kind="ExternalInput")
    mtab_d = nc.dram_tensor("mtab", [128, 12], F32, kind="ExternalInput")
    wq_d = nc.dram_tensor("wqT", [n_layers, H, H], BF16, kind="ExternalInput")
    wk_d = nc.dram_tensor("wkT", [n_layers, H, H], BF16, kind="ExternalInput")
    wv_d = nc.dram_tensor("wvT", [n_layers, H, H], BF16, kind="ExternalInput")
    wo_d = nc.dram_tensor("woT", [n_layers, H, H], BF16, kind="ExternalInput")
    w1_d = nc.dram_tensor("w1T", [n_layers, H, I], BF16, kind="ExternalInput")
    w2_d = nc.dram_tensor("w2T", [n_layers, I, H], BF16, kind="ExternalInput")
    lm_d = nc.dram_tensor("lmT", [H, VS], BF16, kind="ExternalInput")
    out_d = nc.dram_tensor("logits_sh", [B * S, VS], F32, kind="ExternalOutput")

    KS = H * T            # 196608 elems in a K^T / V slab
    KVS = 2 * KS
    kv_in = [nc.dram_tensor(f"kvin{l}", [KVS], BF16) for l in range(n_layers)]
    kv_out = [
        nc.dram_tensor(f"kvout{l}", [4 * KVS], BF16) for l in range(n_layers)
    ]
    hf_in = nc.dram_tensor("hfin", [KS], BF16)
    hf_out = nc.dram_tensor("hfout", [NC_ * KS], BF16, addr_space="Shared")

    KV_GROUPS = [[0, 1, 2, 3], [4, 5, 6, 7]]
    ALL_GROUPS = [list(range(NC_))]

    with tile.TileContext(nc) as tc:
        _trace(tc, n_layers, x0t_d, tri_d, mtab_d, wq_d, wk_d, wv_d, wo_d,
               w1_d, w2_d, lm_d, out_d, kv_in, kv_out, hf_in, hf_out,
               KV_GROUPS, ALL_GROUPS)
    nc.compile()
    return nc


def _trace(tc, n_layers, x0t_d, tri_d, mtab_d, wq_d, wk_d, wv_d, wo_d,
           w1_d, w2_d, lm_d, out_d, kv_in, kv_out, hf_in, hf_out,
           KV_GROUPS, ALL_GROUPS):
    nc = tc.nc
    KS = H * T

    with (
        tc.tile_pool(name="pers", bufs=1) as pers,
        tc.tile_pool(name="psum_g", bufs=2, space="PSUM") as psum_g,
        tc.tile_pool(name="psum_sc", bufs=3, space="PSUM") as psum_sc,
        tc.tile_pool(name="psum_av", bufs=2, space="PSUM") as psum_av,
        tc.tile_pool(name="psum_pb", bufs=1, space="PSUM") as psum_pb,
        tc.tile_pool(name="sm", bufs=2) as sm,
        tc.tile_pool(name="ptp", bufs=8) as ptp,
    ):
        # persistent SBUF state
        x_sb = pers.tile([128, 6 * T], F32, tag="x")          # residual, feat-major
        xb = pers.tile([128, 6 * T], BF16, tag="xb")          # bf16 copy for LN sums
        hln = pers.tile([128, 6 * T], BF16, tag="hln")        # LN out (+ x^2 scratch)
        q_sb = pers.tile([128, 6 * T], BF16, tag="q")         # Q^T local
        kloc = pers.tile([128, 6 * T], BF16, tag="kloc")      # K^T local
        vloc = pers.tile([128, 2 * 780], BF16, tag="vloc")    # V local, 65-interleaved
        ksb = pers.tile([128, 6 * 896], BF16, tag="ksb")      # K^T gathered, blocks 0-6
        vsb = pers.tile([128, 7 * 780], BF16, tag="vsb")      # V gathered, blocks 0-6
        a_sb = pers.tile([128, 6 * T], BF16, tag="a")         # attn out (a^T)
        g_sb = pers.tile([128, 24 * T], BF16, tag="g")        # gelu(FFN1) out
        tri_sb = pers.tile([128, 128], F32, tag="tri")
        mtab = pers.tile([128, 12], F32, tag="mtab")
        ones_k = pers.tile([128, 1], BF16, tag="ok")          # lhsT for col-sums
        ones_b = pers.tile([1, 128], BF16, tag="ob")          # lhsT for broadcasts

        nc.sync.dma_start(tri_sb[:, :], tri_d[:, :])
        nc.sync.dma_start(mtab[:, :], mtab_d[:, :])
        nc.vector.memset(ones_k[:, :], 1.0)
        nc.vector.memset(ones_b[:, :], 1.0)
        # ones columns (col 64 of each 65-wide head slot) for the softmax denom
        nc.vector.memset(
            vloc[:, :].rearrange("p (t h c) -> p t h c", t=2, h=12)[:, :, :, 64:65], 1.0
        )
        nc.vector.memset(
            vsb[:, :].rearrange("p (t h c) -> p t h c", t=7, h=12)[:, :, :, 64:65], 1.0
        )
        for j in range(6):
            nc.sync.dma_start(x_sb[:, T * j : T * (j + 1)], x0t_d[128 * j : 128 * (j + 1), :])

        def layer_norm(wtag):
            """x_sb -> hln (bf16). ln scale/bias pre-folded into weights host-side."""
            nc.scalar.copy(xb[:, :], x_sb[:, :])
            nc.scalar.activation(hln[:, :], xb[:, :], AF.Square)
            ps_s = psum_g.tile([1, T], F32, tag="g")
            ps_q = psum_g.tile([1, T], F32, tag="g")
            for k in range(6):
                nc.tensor.matmul(ps_s[:, :], ones_k[:, :], xb[:, T * k : T * (k + 1)],
                                 start=(k == 0), stop=(k == 5))
            for k in range(6):
                nc.tensor.matmul(ps_q[:, :], ones_k[:, :], hln[:, T * k : T * (k + 1)],
                                 start=(k == 0), stop=(k == 5))
            mean = sm.tile([1, T], F32, tag="mean")
            var = sm.tile([1, T], F32, tag="var")
            rstd = sm.tile([1, T], F32, tag="rstd")
            b0 = sm.tile([1, T], F32, tag="b0")
            rb = sm.tile([1, 2 * T], BF16, tag="rb")
            nc.vector.tensor_scalar_mul(mean[:, :], ps_s[:, :], 1.0 / H)
            nc.vector.tensor_scalar_mul(var[:, :], ps_q[:, :], 1.0 / H)
            nc.vector.tensor_tensor(b0[:, :], mean[:, :], mean[:, :], mybir.AluOpType.mult)
            nc.vector.tensor_sub(var[:, :], var[:, :], b0[:, :])
            nc.vector.tensor_scalar_add(var[:, :], var[:, :], 1e-5)
            nc.scalar.activation(var[:, :], var[:, :], AF.Sqrt)
            nc.vector.reciprocal(rstd[:, :], var[:, :])
            nc.vector.tensor_tensor(b0[:, :], mean[:, :], rstd[:, :], mybir.AluOpType.mult)
            nc.vector.tensor_scalar_mul(b0[:, :], b0[:, :], -1.0)
            nc.scalar.copy(rb[:, 0:T], rstd[:, :])
            nc.scalar.copy(rb[:, T : 2 * T], b0[:, :])
            psb = psum_g.tile([128, 2 * T], F32, tag="g")
            nc.tensor.matmul(psb[:, :], ones_b[:, :], rb[:, :], start=True, stop=True)
            for k in range(6):
                sl = slice(T * k, T * (k + 1))
                nc.vector.tensor_tensor(hln[:, sl], x_sb[:, sl], psb[:, 0:T],
                                        mybir.AluOpType.mult)
                nc.vector.tensor_tensor(hln[:, sl], hln[:, sl], psb[:, T : 2 * T],
                                        mybir.AluOpType.add)

        def gemm_fm(w_t, dout, dst, dst_bf16=True, act=None, add_to_x=False):
            """out^T[dout, T] = W @ act_in^T ; lhsT slabs in w_t [128, 6*dout]."""
            rhs = act if act is not None else hln
            for m in range(dout // 128):
                ps = psum_g.tile([128, T], F32, tag="g")
                nk = w_t.shape[1] // dout
                for k in range(nk):
                    nc.tensor.matmul(
                        ps[:, :],
                        w_t[:, dout * k + 128 * m : dout * k + 128 * (m + 1)],
                        rhs[:, T * k : T * (k + 1)],
                        start=(k == 0), stop=(k == nk - 1),
                    )
                sl = slice(T * m, T * (m + 1))
                if add_to_x:
                    nc.vector.tensor_tensor(x_sb[:, sl], x_sb[:, sl], ps[:, :],
                                            mybir.AluOpType.add)
                elif act is not None and dst_bf16 == "gelu":
                    nc.scalar.activation(dst[:, sl], ps[:, :], AF.Gelu)
                else:
                    nc.scalar.copy(dst[:, sl], ps[:, :])

        with (
            tc.tile_pool(name="wqp", bufs=2) as wqp,
            tc.tile_pool(name="wkp", bufs=2) as wkp,
            tc.tile_pool(name="wvp", bufs=1) as wvp,
            tc.tile_pool(name="wop", bufs=1) as wop,
            tc.tile_pool(name="w1p", bufs=1) as w1p,
            tc.tile_pool(name="w2p", bufs=1) as w2p,
        ):
            for l in range(n_layers):
                wq_t = wqp.tile([128, 6 * H], BF16)
                wk_t = wkp.tile([128, 6 * H], BF16)
                wv_t = wvp.tile([128, 6 * H], BF16)
                wo_t = wop.tile([128, 6 * H], BF16)
                w1_t = w1p.tile([128, 6 * I], BF16)
                w2_t = w2p.tile([128, 24 * H], BF16)
                for k in range(6):
                    r = slice(128 * k, 128 * (k + 1))
                    nc.sync.dma_start(wq_t[:, H * k : H * (k + 1)], wq_d[l, r, :])
                    nc.sync.dma_start(wk_t[:, H * k : H * (k + 1)], wk_d[l, r, :])
                    nc.sync.dma_start(wv_t[:, H * k : H * (k + 1)], wv_d[l, r, :])
                    nc.sync.dma_start(wo_t[:, H * k : H * (k + 1)], wo_d[l, r, :])
                    nc.sync.dma_start(w1_t[:, I * k : I * (k + 1)], w1_d[l, r, :])
                for k in range(24):
                    nc.sync.dma_start(w2_t[:, H * k : H * (k + 1)],
                                      w2_d[l, 128 * k : 128 * (k + 1), :])

                # ---- LN1 + QKV ----
                layer_norm("1")
                gemm_fm(wk_t, H, kloc)
                # V token-major: V[tok, feat] = hln^T chunks as lhsT, wv as rhs
                for tt in range(2):
                    for half in range(2):
                        ps = psum_g.tile([128, 384], F32, tag="g")
                        for k in range(6):
                            nc.tensor.matmul(
                                ps[:, :],
                                hln[:, T * k + 128 * tt : T * k + 128 * (tt + 1)],
                                wv_t[:, H * k + 384 * half : H * k + 384 * (half + 1)],
                                start=(k == 0), stop=(k == 5),
                            )
                        dst = vloc[:, 780 * tt : 780 * (tt + 1)].rearrange(
                            "p (h c) -> p h c", c=65)[:, 6 * half : 6 * (half + 1), 0:64]
                        nc.scalar.copy(dst, ps[:, :].rearrange("p (h c) -> p h c", c=64))
                gemm_fm(wq_t, H, q_sb)

                # ---- ship K/V into the AG bounce, run AllGather ----
                kv_k = kv_in[l].ap()[0:KS].rearrange("(p f) -> p f", p=H)
                kv_v = kv_in[l].ap()[KS : 2 * KS].rearrange("(p f) -> p f", p=T)
                for m in range(6):
                    nc.sync.dma_start(kv_k[128 * m : 128 * (m + 1), :],
                                      kloc[:, T * m : T * (m + 1)])
                for tt in range(2):
                    src = vloc[:, 780 * tt : 780 * (tt + 1)].rearrange(
                        "p (h c) -> p h c", c=65)[:, :, 0:64]
                    nc.sync.dma_start(
                        kv_v[128 * tt : 128 * (tt + 1), :].rearrange(
                            "p (h c) -> p h c", c=64), src)
                nc.gpsimd.collective_compute(
                    "AllGather", mybir.AluOpType.bypass, replica_groups=KV_GROUPS,
                    ins=[kv_in[l].ap().opt()], outs=[kv_out[l].ap().opt()])

                # ---- load gathered K/V (global k-blocks 0..6) ----
                for r in range(4):
                    slab_k = kv_out[l].ap()[2 * KS * r : 2 * KS * r + KS].rearrange(
                        "(p f) -> p f", p=H)
                    slab_v = kv_out[l].ap()[2 * KS * r + KS : 2 * KS * (r + 1)].rearrange(
                        "(p f) -> p f", p=T)
                    for j in range(6):
                        src = slab_k[128 * j : 128 * (j + 1), :]
                        for half, t in ((0, r), (1, 7 - r)):
                            if t == 7:
                                continue
                            nc.sync.dma_start(
                                ksb[:, 896 * j + 128 * t : 896 * j + 128 * (t + 1)],
                                src[:, 128 * half : 128 * (half + 1)])
                    for half in range(2):
                        t = r if half == 0 else 7 - r
                        if t == 7:
                            continue
                        dst = vsb[:, 780 * t : 780 * (t + 1)].rearrange(
                            "p (h c) -> p h c", c=65)[:, :, 0:64]
                        nc.sync.dma_start(
                            dst, slab_v[128 * half : 128 * (half + 1), :].rearrange(
                                "p (h c) -> p h c", c=64))

                # ---- attention: 12 static slots per head ----
                for h in range(NH):
                    jh, po = h // 2, (h % 2) * 64
                    avL = psum_av.tile([65, 128], F32, tag="av")
                    avH = psum_av.tile([65, 128], F32, tag="av")
                    # (slot, is_diag, qt, av_psum, av_start, av_stop, k_lhsT, v_lhsT)
                    slots = []
                    for s in range(12):
                        if s < 2:
                            qt = s
                            klh = kloc[po : po + 64, T * jh + 128 * qt : T * jh + 128 * (qt + 1)]
                            vlh = vloc[:, 780 * qt + 65 * h : 780 * qt + 65 * (h + 1)]
                            diag = True
                        elif s < 5:
                            qt, j = 0, s - 2
                            klh = ksb[po : po + 64, 896 * jh + 128 * j : 896 * jh + 128 * (j + 1)]
                            vlh = vsb[:, 780 * j + 65 * h : 780 * j + 65 * (h + 1)]
                            diag = False
                        else:
                            qt, j = 1, s - 5
                            klh = ksb[po : po + 64, 896 * jh + 128 * j : 896 * jh + 128 * (j + 1)]
                            vlh = vsb[:, 780 * j + 65 * h : 780 * j + 65 * (h + 1)]
                            diag = False
                        slots.append((s, diag, qt, klh, vlh))
                    last_for = {0: 4, 1: 11}
                    for s, diag, qt, klh, vlh in slots:
                        qs = q_sb[po : po + 64, T * jh + 128 * qt : T * jh + 128 * (qt + 1)]
                        ps = psum_sc.tile([128, 128], F32, tag="sc")
                        nc.tensor.matmul(ps[:, :], klh, qs, start=True, stop=True)
                        if diag:
                            nc.vector.tensor_tensor(ps[:, :], ps[:, :], tri_sb[:, :],
                                                    mybir.AluOpType.add)
                        p_t = ptp.tile([128, 128], BF16, tag="pt")
                        nc.scalar.activation(p_t[:, :], ps[:, :], AF.Exp,
                                             bias=mtab[:, s : s + 1])
                        av = avL if qt == 0 else avH
                        nc.tensor.matmul(av[:, :], vlh, p_t[:, :],
                                         start=(s == qt), stop=(s == last_for[qt]))
                    # softmax denominators -> divide -> a_sb
                    recip = sm.tile([1, T], F32, tag="recip")
                    nc.vector.reciprocal(recip[:, 0:128], avL[64:65, :])
                    nc.vector.reciprocal(recip[:, 128:256], avH[64:65, :])
                    rq = sm.tile([1, T], BF16, tag="rq")
                    nc.scalar.copy(rq[:, :], recip[:, :])
                    pb = psum_pb.tile([64, T], F32, tag="pb")
                    nc.tensor.matmul(pb[:, :], ones_b[:, 0:64], rq[:, :],
                                     start=True, stop=True)
                    rb2 = sm.tile([64, T], BF16, tag="rb2")
                    nc.scalar.copy(rb2[:, :], pb[:, :])
                    asl = a_sb[po : po + 64, T * jh : T * (jh + 1)]
                    nc.vector.tensor_tensor(asl[:, 0:128], avL[0:64, :], rb2[:, 0:128],
                                            mybir.AluOpType.mult)
                    nc.vector.tensor_tensor(asl[:, 128:256], avH[0:64, :], rb2[:, 128:256],
                                            mybir.AluOpType.mult)

                # ---- O proj (+ residual), LN2, FFN (+ residual) ----
                gemm_fm(wo_t, H, None, act=a_sb, add_to_x=True)
                layer_norm("2")
                gemm_fm(w1_t, I, g_sb, act=hln, dst_bf16="gelu")
                gemm_fm(w2_t, H, None, act=g_sb, add_to_x=True)

        # ---- final LN, AllGather h, vocab-sharded lm_head ----
        layer_norm("f")
        for m in range(6):
            nc.sync.dma_start(
                hf_in.ap().rearrange("(p f) -> p f", p=H)[128 * m : 128 * (m + 1), :],
                hln[:, T * m : T * (m + 1)])
        nc.gpsimd.collective_compute(
            "AllGather", mybir.AluOpType.bypass, replica_groups=ALL_GROUPS,
            ins=[hf_in.ap().opt()], outs=[hf_out.ap().opt()])

        with (
            tc.tile_pool(name="hsb", bufs=1) as hsbp,
            tc.tile_pool(name="lmw", bufs=12) as lmwp,
            tc.tile_pool(name="lo", bufs=4) as lop,
        ):
            h_sb = hsbp.tile([128, 6 * 2048], BF16)
            for r in range(NC_):
                g, c = r // 4, r % 4
                slab = hf_out.ap()[KS * r : KS * (r + 1)].rearrange("(p f) -> p f", p=H)
                for j in range(6):
                    for half, gt in ((0, c), (1, 7 - c)):
                        col = 1024 * g + 128 * gt
                        nc.sync.dma_start(
                            h_sb[:, 2048 * j + col : 2048 * j + col + 128],
                            slab[128 * j : 128 * (j + 1), 128 * half : 128 * (half + 1)])
            nv_sizes = [512] * 12 + [VS - 12 * 512]
            for v in range(13):
                nv = nv_sizes[v]
                lw = []
                for k in range(6):
                    t = lmwp.tile([128, 512], BF16, tag="lw")
                    nc.sync.dma_start(t[:, 0:nv], lm_d[128 * k : 128 * (k + 1),
                                                       512 * v : 512 * v + nv])
                    lw.append(t)
                for m in range(16):
                    ps = psum_g.tile([128, 512], F32, tag="g")
                    for k in range(6):
                        nc.tensor.matmul(
                            ps[:, 0:nv],
                            h_sb[:, 2048 * k + 128 * m : 2048 * k + 128 * (m + 1)],
                            lw[k][:, 0:nv], start=(k == 0), stop=(k == 5))
                    lo = lop.tile([128, 512], F32, tag="lo")
                    nc.scalar.copy(lo[:, 0:nv], ps[:, 0:nv])
                    nc.sync.dma_start(out_d[128 * m : 128 * (m + 1), 512 * v : 512 * v + nv],
                                      lo[:, 0:nv])


def kernel(input_ids, attention_mask, tok_emb, pos_emb, ln1_w, ln1_b, qw, kw, vw,
           ow, ob, ln2_w, ln2_b, w1, b1, w2, b2, lnf_w, lnf_b, lm_head_w):
    n_layers = int(qw.shape[0])
    for b in (ln1_b, ob, ln2_b, b1, b2, lnf_b):
        assert np.abs(np.asarray(b)).max() == 0.0, "nonzero biases unsupported"

    input_ids = np.asarray(input_ids)
    x0 = np.asarray(tok_emb)[input_ids] + np.asarray(pos_emb)[:S][None]  # [B,S,H] f32

    scale = 1.0 / np.sqrt(HD)
    ln1_w = np.asarray(ln1_w, np.float32)
    ln2_w = np.asarray(ln2_w, np.float32)
    wqT = np.ascontiguousarray(
        np.transpose(np.asarray(qw), (0, 2, 1)) * ln1_w[:, :, None] * scale).astype(BF16_NP)
    wkT = np.ascontiguousarray(
        np.transpose(np.asarray(kw), (0, 2, 1)) * ln1_w[:, :, None]).astype(BF16_NP)
    wvT = np.ascontiguousarray(
        np.transpose(np.asarray(vw), (0, 2, 1)) * ln1_w[:, :, None]).astype(BF16_NP)
    woT = np.ascontiguousarray(np.transpose(np.asarray(ow), (0, 2, 1))).astype(BF16_NP)
    w1T = np.ascontiguousarray(
        np.transpose(np.asarray(w1), (0, 2, 1)) * ln2_w[:, :, None]).astype(BF16_NP)
    w2T = np.ascontiguousarray(np.transpose(np.asarray(w2), (0, 2, 1))).astype(BF16_NP)
    lm_pad = np.zeros((VPAD, H), np.float32)
    lm_pad[:V] = np.asarray(lm_head_w) * np.asarray(lnf_w, np.float32)[None, :]

    tri = np.where(np.arange(128)[:, None] <= np.arange(128)[None, :], 0.0, NEG
                   ).astype(np.float32)
    am = np.asarray(attention_mask)
    kbias = np.where(am != 0, 0.0, NEG).astype(np.float32)  # [B,S] key-pad bias

    in_maps = []
    for cid in range(NC_):
        g, c = cid // 4, cid % 4
        tiles = (c, 7 - c)
        x0t = np.ascontiguousarray(np.concatenate(
            [x0[g, 128 * t : 128 * (t + 1)] for t in tiles], axis=0).T.astype(np.float32))
        mtab = np.zeros((128, 12), np.float32)
        mtab[:, 0] = kbias[g, 128 * c : 128 * (c + 1)]
        mtab[:, 1] = kbias[g, 128 * (7 - c) : 128 * (8 - c)]
        for j in range(3):
            mtab[:, 2 + j] = (kbias[g, 128 * j : 128 * (j + 1)] if j < c else NEG)
        for j in range(7):
            mtab[:, 5 + j] = (kbias[g, 128 * j : 128 * (j + 1)] if j <= 6 - c else NEG)
        lmT = np.ascontiguousarray(lm_pad[VS * cid : VS * (cid + 1)].T).astype(BF16_NP)
        in_maps.append({
            "x0t": x0t, "tri": tri, "mtab": mtab,
            "wqT": wqT, "wkT": wkT, "wvT": wvT, "woT": woT, "w1T": w1T, "w2T": w2T,
            "lmT": lmT,
        })

    if n_layers not in _prog_cache:
        _prog_cache[n_layers] = _build_program(n_layers)
    nc = _prog_cache[n_layers]
    import os
    _trace = bool(os.environ.get("KTRACE"))
    res = None
    for attempt in range(3):
        try:
            res = run_bass_kernel_spmd(nc, in_maps, core_ids=list(range(NC_)),
                                       trace=_trace and attempt == 0)
            break
        except ModuleNotFoundError:
            _trace = False
        except Exception:
            if attempt == 2:
                raise
    if res is None:
        res = run_bass_kernel_spmd(nc, in_maps, core_ids=list(range(NC_)))
    global last_exec_time_ns
    last_exec_time_ns = res.exec_time_ns
    full = np.concatenate([res.results[c]["logits_sh"] for c in range(NC_)], axis=1)
    return np.ascontiguousarray(full[:, :V].reshape(B, S, V).astype(np.float32))


# revision 10
# speedup vs baseline: 1.2112x; 1.0186x over previous
"""GPT-2-small forward pass on 8 Trainium2 NeuronCores.

Sharding: 2 data-parallel groups of 4 cores (one per batch element).  Within a
group each core owns 256 tokens (query tiles {c, 7-c} of the 8x128-token tiles,
mirrored pairing so every core has exactly 9 causal k-blocks of attention work).
Per layer the only communication is one 4-rank AllGather of the local K^T/V
slabs.  FFN / LN / residual are fully local with replicated bf16 weights.
The lm_head is vocab-sharded 8 ways after a final 8-rank AllGather of the
hidden states; the host concatenates the logit shards.

Attention is a fully static 12-slot schedule per head (uniform SPMD program):
slots 0/1 are the two diagonal blocks (read from the core's *local* K/V copy at
static offsets, triangular mask added in PSUM), slots 2-4 are off-diagonal
k-blocks 0-2 for the low query tile, slots 5-11 are k-blocks 0-6 for the high
tile.  Unused slots are disabled via a per-core [128,12] bias table that feeds
the exp's ACT bias (-1e9 -> exp==0), so per-core causal asymmetry is pure data.
"""

import os
import sys

sys.path.insert(0, "/opt/trn_rl_repo")
os.environ.setdefault("NEURON_RT_EXEC_TIMEOUT", "600")

import numpy as np
import ml_dtypes

import concourse.bass as bass
import concourse.bacc as bacc
import concourse.mybir as mybir
import concourse.tile as tile
import concourse.tile_utils as tile_utils
from concourse.bass_utils import run_bass_kernel_spmd

BF16_NP = ml_dtypes.bfloat16
F32 = mybir.dt.float32
BF16 = mybir.dt.bfloat16
AF = mybir.ActivationFunctionType

V, H, L, NH, I, PMAX = 50257, 768, 12, 12, 3072, 2048
B, S = 2, 1024
HD = H // NH          # 64
T = 256               # local tokens per core
NC_ = 8               # cores
VS = 6283             # vocab shard per core (8*6283 = 50264 >= 50257, zero-padded)
VPAD = NC_ * VS
NEG = -1e9

# lift tile's stale SBUF cap (cayman has 208KB/partition usable)
tile_utils.max_sbuf_usage = 204 * 1024

_prog_cache = {}
last_exec_time_ns = None


def _build_program(n_layers):
    nc = bacc.Bacc("TRN2", target_bir_lowering=False, debug=False, num_devices=NC_)

    # ---- DRAM I/O ----
    x0t_d = nc.dram_tensor("x0t", [H, T], F32, kind="ExternalInput")
    tri_d = nc.dram_tensor("tri", [128, 128], F32, kind="ExternalInput")
    mtab_d = nc.dram_tensor("mtab", [128, 12], F32, kind="ExternalInput")
    wq_d = nc.dram_tensor("wqT", [n_layers, H, H], BF16, kind="ExternalInput")
    wk_d = nc.dram_tensor("wkT", [n_layers, H, H], BF16, kind="ExternalInput")
    wv_d = nc.dram_tensor("wvT", [n_layers, H, H], BF16, kind="ExternalInput")
    wo_d = nc.dram_tensor("woT", [n_layers, H, H], BF16, kind="ExternalInput")
    w1_d = nc.dram_tensor("w1T", [n_layers, H, I], BF16, kind="ExternalInput")
    w2_d = nc.dram_tensor("w2T", [n_layers, I, H], BF16, kind="ExternalInput")
    lm_d = nc.dram_tensor("lmT", [H, VS], BF16, kind="ExternalInput")
    out_d = nc.dram_tensor("logits_sh", [B * S, VS], F32, kind="ExternalOutput")

    KS = H * T            # 196608 elems in a K^T / V slab
    KVS = 2 * KS
    kv_in = [nc.dram_tensor(f"kvin{l}", [KVS], BF16) for l in range(n_layers)]
    kv_out = [
        nc.dram_tensor(f"kvout{l}", [4 * KVS], BF16) for l in range(n_layers)
    ]
    hf_in = nc.dram_tensor("hfin", [KS], BF16)
    hf_out = nc.dram_tensor("hfout", [NC_ * KS], BF16, addr_space="Shared")

    KV_GROUPS = [[0, 1, 2, 3], [4, 5, 6, 7]]
    ALL_GROUPS = [list(range(NC_))]

    with tile.TileContext(nc) as tc:
        _trace(tc, n_layers, x0t_d, tri_d, mtab_d, wq_d, wk_d, wv_d, wo_d,
               w1_d, w2_d, lm_d, out_d, kv_in, kv_out, hf_in, hf_out,
               KV_GROUPS, ALL_GROUPS)
    nc.compile()
    return nc


def _trace(tc, n_layers, x0t_d, tri_d, mtab_d, wq_d, wk_d, wv_d, wo_d,
           w1_d, w2_d, lm_d, out_d, kv_in, kv_out, hf_in, hf_out,
           KV_GROUPS, ALL_GROUPS):
    nc = tc.nc
    KS = H * T

    with (
        tc.tile_pool(name="pers", bufs=1) as pers,
        tc.tile_pool(name="psum_g", bufs=2, space="PSUM") as psum_g,
        tc.tile_pool(name="psum_sc", bufs=2, space="PSUM") as psum_sc,
        tc.tile_pool(name="psum_av", bufs=3, space="PSUM") as psum_av,
        tc.tile_pool(name="psum_pb", bufs=1, space="PSUM") as psum_pb,
        tc.tile_pool(name="sm", bufs=2) as sm,
        tc.tile_pool(name="ptp", bufs=8) as ptp,
    ):
        # persistent SBUF state
        x_sb = pers.tile([128, 6 * T], F32, tag="x")          # residual, feat-major
        xb = pers.tile([128, 6 * T], BF16, tag="xb")          # bf16 copy for LN sums
        hln = pers.tile([128, 6 * T], BF16, tag="hln")        # LN out (+ x^2 scratch)
        q_sb = pers.tile([128, 6 * T], BF16, tag="q")         # Q^T local
        kloc = pers.tile([128, 6 * T], BF16, tag="kloc")      # K^T local
        vloc = pers.tile([128, 2 * 780], BF16, tag="vloc")    # V local, 65-interleaved
        ksb = pers.tile([128, 6 * 896], BF16, tag="ksb")      # K^T gathered, blocks 0-6
        vsb = pers.tile([128, 7 * 780], BF16, tag="vsb")      # V gathered, blocks 0-6
        a_sb = pers.tile([128, 6 * T], BF16, tag="a")         # attn out (a^T)
        g_sb = pers.tile([128, 24 * T], BF16, tag="g")        # gelu(FFN1) out
        tri_sb = pers.tile([128, 128], F32, tag="tri")
        mtab = pers.tile([128, 12], F32, tag="mtab")
        ones_k = pers.tile([128, 1], BF16, tag="ok")          # lhsT for col-sums
        ones_b = pers.tile([1, 128], BF16, tag="ob")          # lhsT for broadcasts

        nc.sync.dma_start(tri_sb[:, :], tri_d[:, :])
        nc.sync.dma_start(mtab[:, :], mtab_d[:, :])
        nc.vector.memset(ones_k[:, :], 1.0)
        nc.vector.memset(ones_b[:, :], 1.0)
        # ones columns (col 64 of each 65-wide head slot) for the softmax denom
        nc.vector.memset(
            vloc[:, :].rearrange("p (t h c) -> p t h c", t=2, h=12)[:, :, :, 64:65], 1.0
        )
        nc.vector.memset(
            vsb[:, :].rearrange("p (t h c) -> p t h c", t=7, h=12)[:, :, :, 64:65], 1.0
        )
        for j in range(6):
            nc.sync.dma_start(x_sb[:, T * j : T * (j + 1)], x0t_d[128 * j : 128 * (j + 1), :])

        def layer_norm(wtag):
            """x_sb -> hln (bf16). ln scale/bias pre-folded into weights host-side."""
            nc.scalar.copy(xb[:, :], x_sb[:, :])
            nc.scalar.activation(hln[:, :], xb[:, :], AF.Square)
            ps_s = psum_g.tile([1, T], F32, tag="g")
            ps_q = psum_g.tile([1, T], F32, tag="g")
            for k in range(6):
                nc.tensor.matmul(ps_s[:, :], ones_k[:, :], xb[:, T * k : T * (k + 1)],
                                 start=(k == 0), stop=(k == 5))
            for k in range(6):
                nc.tensor.matmul(ps_q[:, :], ones_k[:, :], hln[:, T * k : T * (k + 1)],
                                 start=(k == 0), stop=(k == 5))
            mean = sm.tile([1, T], F32, tag="mean")
            var = sm.tile([1, T], F32, tag="var")
            rstd = sm.tile([1, T], F32, tag="rstd")
            b0 = sm.tile([1, T], F32, tag="b0")
            rb = sm.tile([1, 2 * T], BF16, tag="rb")
            nc.vector.tensor_scalar_mul(mean[:, :], ps_s[:, :], 1.0 / H)
            nc.vector.tensor_scalar_mul(var[:, :], ps_q[:, :], 1.0 / H)
            nc.vector.tensor_tensor(b0[:, :], mean[:, :], mean[:, :], mybir.AluOpType.mult)
            nc.vector.tensor_sub(var[:, :], var[:, :], b0[:, :])
            nc.vector.tensor_scalar_add(var[:, :], var[:, :], 1e-5)
            nc.scalar.activation(var[:, :], var[:, :], AF.Sqrt)
            nc.vector.reciprocal(rstd[:, :], var[:, :])
            nc.vector.tensor_tensor(b0[:, :], mean[:, :], rstd[:, :], mybir.AluOpType.mult)
            nc.vector.tensor_scalar_mul(b0[:, :], b0[:, :], -1.0)
            nc.scalar.copy(rb[:, 0:T], rstd[:, :])
            nc.scalar.copy(rb[:, T : 2 * T], b0[:, :])
            psb = psum_g.tile([128, 2 * T], F32, tag="g")
            nc.tensor.matmul(psb[:, :], ones_b[:, :], rb[:, :], start=True, stop=True)
            for k in range(6):
                sl = slice(T * k, T * (k + 1))
                nc.vector.tensor_tensor(hln[:, sl], x_sb[:, sl], psb[:, 0:T],
                                        mybir.AluOpType.mult)
                nc.vector.tensor_tensor(hln[:, sl], hln[:, sl], psb[:, T : 2 * T],
                                        mybir.AluOpType.add)

        def gemm_fm(w_t, dout, dst, dst_bf16=True, act=None, add_to_x=False):
            """out^T[dout, T] = W @ act_in^T ; lhsT slabs in w_t [128, 6*dout]."""
            rhs = act if act is not None else hln
            for m in range(dout // 128):
                ps = psum_g.tile([128, T], F32, tag="g")
                nk = w_t.shape[1] // dout
                for k in range(nk):
                    nc.tensor.matmul(
                        ps[:, :],
                        w_t[:, dout * k + 128 * m : dout * k + 128 * (m + 1)],
                        rhs[:, T * k : T * (k + 1)],
                        start=(k == 0), stop=(k == nk - 1),
                    )
                sl = slice(T * m, T * (m + 1))
                if add_to_x:
                    nc.vector.tensor_tensor(x_sb[:, sl], x_sb[:, sl], ps[:, :],
                                            mybir.AluOpType.add)
                elif act is not None and dst_bf16 == "gelu":
                    nc.scalar.activation(dst[:, sl], ps[:, :], AF.Gelu)
                else:
                    nc.scalar.copy(dst[:, sl], ps[:, :])

        with (
            tc.tile_pool(name="wqp", bufs=2) as wqp,
            tc.tile_pool(name="wkp", bufs=2) as wkp,
            tc.tile_pool(name="wvp", bufs=1) as wvp,
            tc.tile_pool(name="wop", bufs=1) as wop,
            tc.tile_pool(name="w1p", bufs=1) as w1p,
            tc.tile_pool(name="w2p", bufs=1) as w2p,
        ):
            for l in range(n_layers):
                wq_t = wqp.tile([128, 6 * H], BF16)
                wk_t = wkp.tile([128, 6 * H], BF16)
                wv_t = wvp.tile([128, 6 * H], BF16)
                wo_t = wop.tile([128, 6 * H], BF16)
                w1_t = w1p.tile([128, 6 * I], BF16)
                w2_t = w2p.tile([128, 24 * H], BF16)
                for k in range(6):
                    r = slice(128 * k, 128 * (k + 1))
                    nc.sync.dma_start(wq_t[:, H * k : H * (k + 1)], wq_d[l, r, :])
                    nc.sync.dma_start(wk_t[:, H * k : H * (k + 1)], wk_d[l, r, :])
                    nc.sync.dma_start(wv_t[:, H * k : H * (k + 1)], wv_d[l, r, :])
                    nc.sync.dma_start(wo_t[:, H * k : H * (k + 1)], wo_d[l, r, :])
                    nc.sync.dma_start(w1_t[:, I * k : I * (k + 1)], w1_d[l, r, :])
                for k in range(24):
                    nc.sync.dma_start(w2_t[:, H * k : H * (k + 1)],
                                      w2_d[l, 128 * k : 128 * (k + 1), :])

                # ---- LN1 + QKV ----
                layer_norm("1")
                gemm_fm(wk_t, H, kloc)
                # V token-major: V[tok, feat] = hln^T chunks as lhsT, wv as rhs
                for tt in range(2):
                    for half in range(2):
                        ps = psum_g.tile([128, 384], F32, tag="g")
                        for k in range(6):
                            nc.tensor.matmul(
                                ps[:, :],
                                hln[:, T * k + 128 * tt : T * k + 128 * (tt + 1)],
                                wv_t[:, H * k + 384 * half : H * k + 384 * (half + 1)],
                                start=(k == 0), stop=(k == 5),
                            )
                        dst = vloc[:, 780 * tt : 780 * (tt + 1)].rearrange(
                            "p (h c) -> p h c", c=65)[:, 6 * half : 6 * (half + 1), 0:64]
                        nc.scalar.copy(dst, ps[:, :].rearrange("p (h c) -> p h c", c=64))
                gemm_fm(wq_t, H, q_sb)

                # ---- ship K/V into the AG bounce, run AllGather ----
                kv_k = kv_in[l].ap()[0:KS].rearrange("(p f) -> p f", p=H)
                kv_v = kv_in[l].ap()[KS : 2 * KS].rearrange("(p f) -> p f", p=T)
                for m in range(6):
                    nc.sync.dma_start(kv_k[128 * m : 128 * (m + 1), :],
                                      kloc[:, T * m : T * (m + 1)])
                for tt in range(2):
                    src = vloc[:, 780 * tt : 780 * (tt + 1)].rearrange(
                        "p (h c) -> p h c", c=65)[:, :, 0:64]
                    nc.sync.dma_start(
                        kv_v[128 * tt : 128 * (tt + 1), :].rearrange(
                            "p (h c) -> p h c", c=64), src)
                nc.gpsimd.collective_compute(
                    "AllGather", mybir.AluOpType.bypass, replica_groups=KV_GROUPS,
                    ins=[kv_in[l].ap().opt()], outs=[kv_out[l].ap().opt()])

                # ---- load gathered K/V (global k-blocks 0..6) ----
                for r in range(4):
                    slab_k = kv_out[l].ap()[2 * KS * r : 2 * KS * r + KS].rearrange(
                        "(p f) -> p f", p=H)
                    slab_v = kv_out[l].ap()[2 * KS * r + KS : 2 * KS * (r + 1)].rearrange(
                        "(p f) -> p f", p=T)
                    for j in range(6):
                        src = slab_k[128 * j : 128 * (j + 1), :]
                        for half, t in ((0, r), (1, 7 - r)):
                            if t == 7:
                                continue
                            nc.sync.dma_start(
                                ksb[:, 896 * j + 128 * t : 896 * j + 128 * (t + 1)],
                                src[:, 128 * half : 128 * (half + 1)])
                    for half in range(2):
                        t = r if half == 0 else 7 - r
                        if t == 7:
                            continue
                        dst = vsb[:, 780 * t : 780 * (t + 1)].rearrange(
                            "p (h c) -> p h c", c=65)[:, :, 0:64]
                        nc.sync.dma_start(
                            dst, slab_v[128 * half : 128 * (half + 1), :].rearrange(
                                "p (h c) -> p h c", c=64))

                # ---- attention: 12 static slots per head ----
                for h in range(NH):
                    jh, po = h // 2, (h % 2) * 64
                    avL = psum_av.tile([65, 128], F32, tag="av")
                    avH = psum_av.tile([65, 128], F32, tag="av")
                    # (slot, is_diag, qt, av_psum, av_start, av_stop, k_lhsT, v_lhsT)
                    slots = []
                    for s in range(12):
                        if s < 2:
                            qt = s
                            klh = kloc[po : po + 64, T * jh + 128 * qt : T * jh + 128 * (qt + 1)]
                            vlh = vloc[:, 780 * qt + 65 * h : 780 * qt + 65 * (h + 1)]
                            diag = True
                        elif s < 5:
                            qt, j = 0, s - 2
                            klh = ksb[po : po + 64, 896 * jh + 128 * j : 896 * jh + 128 * (j + 1)]
                            vlh = vsb[:, 780 * j + 65 * h : 780 * j + 65 * (h + 1)]
                            diag = False
                        else:
                            qt, j = 1, s - 5
                            klh = ksb[po : po + 64, 896 * jh + 128 * j : 896 * jh + 128 * (j + 1)]
                            vlh = vsb[:, 780 * j + 65 * h : 780 * j + 65 * (h + 1)]
                            diag = False
                        slots.append((s, diag, qt, klh, vlh))
                    last_for = {0: 4, 1: 11}
                    for s, diag, qt, klh, vlh in slots:
                        qs = q_sb[po : po + 64, T * jh + 128 * qt : T * jh + 128 * (qt + 1)]
                        ps = psum_sc.tile([128, 128], F32, tag="sc")
                        nc.tensor.matmul(ps[:, :], klh, qs, start=True, stop=True)
                        if diag:
                            nc.vector.tensor_tensor(ps[:, :], ps[:, :], tri_sb[:, :],
                                                    mybir.AluOpType.add)
                        p_t = ptp.tile([128, 128], BF16, tag="pt")
                        nc.scalar.activation(p_t[:, :], ps[:, :], AF.Exp,
                                             bias=mtab[:, s : s + 1])
                        av = avL if qt == 0 else avH
                        nc.tensor.matmul(av[:, :], vlh, p_t[:, :],
                                         start=(s == qt), stop=(s == last_for[qt]))
                    # softmax denominators -> divide -> a_sb
                    recip = sm.tile([1, T], F32, tag="recip")
                    nc.vector.reciprocal(recip[:, 0:128], avL[64:65, :])
                    nc.vector.reciprocal(recip[:, 128:256], avH[64:65, :])
                    rq = sm.tile([1, T], BF16, tag="rq")
                    nc.scalar.copy(rq[:, :], recip[:, :])
                    pb = psum_pb.tile([64, T], F32, tag="pb")
                    nc.tensor.matmul(pb[:, :], ones_b[:, 0:64], rq[:, :],
                                     start=True, stop=True)
                    rb2 = sm.tile([64, T], BF16, tag="rb2")
                    nc.scalar.copy(rb2[:, :], pb[:, :])
                    asl = a_sb[po : po + 64, T * jh : T * (jh + 1)]
                    nc.vector.tensor_tensor(asl[:, 0:128], avL[0:64, :], rb2[:, 0:128],
                                            mybir.AluOpType.mult)
                    nc.vector.tensor_tensor(asl[:, 128:256], avH[0:64, :], rb2[:, 128:256],
                                            mybir.AluOpType.mult)

                # ---- O proj (+ residual), LN2, FFN (+ residual) ----
                gemm_fm(wo_t, H, None, act=a_sb, add_to_x=True)
                layer_norm("2")
                gemm_fm(w1_t, I, g_sb, act=hln, dst_bf16="gelu")
                gemm_fm(w2_t, H, None, act=g_sb, add_to_x=True)

        # ---- final LN, AllGather h, vocab-sharded lm_head ----
        layer_norm("f")
        for m in range(6):
            nc.sync.dma_start(
                hf_in.ap().rearrange("(p f) -> p f", p=H)[128 * m : 128 * (m + 1), :],
                hln[:, T * m : T * (m + 1)])
        nc.gpsimd.collective_compute(
            "AllGather", mybir.AluOpType.bypass, replica_groups=ALL_GROUPS,
            ins=[hf_in.ap().opt()], outs=[hf_out.ap().opt()])

        with (
            tc.tile_pool(name="hsb", bufs=1) as hsbp,
            tc.tile_pool(name="lmw", bufs=12) as lmwp,
            tc.tile_pool(name="lo", bufs=4) as lop,
        ):
            h_sb = hsbp.tile([128, 6 * 2048], BF16)
            for r in range(NC_):
                g, c = r // 4, r % 4
                slab = hf_out.ap()[KS * r : KS * (r + 1)].rearrange("(p f) -> p f", p=H)
                for j in range(6):
                    for half, gt in ((0, c), (1, 7 - c)):
                        col = 1024 * g + 128 * gt
                        nc.sync.dma_start(
                            h_sb[:, 2048 * j + col : 2048 * j + col + 128],
                            slab[128 * j : 128 * (j + 1), 128 * half : 128 * (half + 1)])
            nv_sizes = [512] * 12 + [VS - 12 * 512]
            for v in range(13):
                nv = nv_sizes[v]
                lw = []
                for k in range(6):
                    t = lmwp.tile([128, 512], BF16, tag="lw")
                    nc.sync.dma_start(t[:, 0:nv], lm_d[128 * k : 128 * (k + 1),
                                                       512 * v : 512 * v + nv])
                    lw.append(t)
                for m in range(16):
                    ps = psum_g.tile([128, 512], F32, tag="g")
                    for k in range(6):
                        nc.tensor.matmul(
                            ps[:, 0:nv],
                            h_sb[:, 2048 * k + 128 * m : 2048 * k + 128 * (m + 1)],
                            lw[k][:, 0:nv], start=(k == 0), stop=(k == 5))
                    lo = lop.tile([128, 512], F32, tag="lo")
                    nc.scalar.copy(lo[:, 0:nv], ps[:, 0:nv])
                    nc.sync.dma_start(out_d[128 * m : 128 * (m + 1), 512 * v : 512 * v + nv],
                                      lo[:, 0:nv])


def kernel(input_ids, attention_mask, tok_emb, pos_emb, ln1_w, ln1_b, qw, kw, vw,
           ow, ob, ln2_w, ln2_b, w1, b1, w2, b2, lnf_w, lnf_b, lm_head_w):
    n_layers = int(qw.shape[0])
    for b in (ln1_b, ob, ln2_b, b1, b2, lnf_b):
        assert np.abs(np.asarray(b)).max() == 0.0, "nonzero biases unsupported"

    input_ids = np.asarray(input_ids)
    x0 = np.asarray(tok_emb)[input_ids] + np.asarray(pos_emb)[:S][None]  # [B,S,H] f32

    scale = 1.0 / np.sqrt(HD)
    ln1_w = np.asarray(ln1_w, np.float32)
    ln2_w = np.asarray(ln2_w, np.float32)
    wqT = np.ascontiguousarray(
        np.transpose(np.asarray(qw), (0, 2, 1)) * ln1_w[:, :, None] * scale).astype(BF16_NP)
    wkT = np.ascontiguousarray(
        np.transpose(np.asarray(kw), (0, 2, 1)) * ln1_w[:, :, None]).astype(BF16_NP)
    wvT = np.ascontiguousarray(
        np.transpose(np.asarray(vw), (0, 2, 1)) * ln1_w[:, :, None]).astype(BF16_NP)
    woT = np.ascontiguousarray(np.transpose(np.asarray(ow), (0, 2, 1))).astype(BF16_NP)
    w1T = np.ascontiguousarray(
        np.transpose(np.asarray(w1), (0, 2, 1)) * ln2_w[:, :, None]).astype(BF16_NP)
    w2T = np.ascontiguousarray(np.transpose(np.asarray(w2), (0, 2, 1))).astype(BF16_NP)
    lm_pad = np.zeros((VPAD, H), np.float32)
    lm_pad[:V] = np.asarray(lm_head_w) * np.asarray(lnf_w, np.float32)[None, :]

    tri = np.where(np.arange(128)[:, None] <= np.arange(128)[None, :], 0.0, NEG
                   ).astype(np.float32)
    am = np.asarray(attention_mask)
    kbias = np.where(am != 0, 0.0, NEG).astype(np.float32)  # [B,S] key-pad bias

    in_maps = []
    for cid in range(NC_):
        g, c = cid // 4, cid % 4
        tiles = (c, 7 - c)
        x0t = np.ascontiguousarray(np.concatenate(
            [x0[g, 128 * t : 128 * (t + 1)] for t in tiles], axis=0).T.astype(np.float32))
        mtab = np.zeros((128, 12), np.float32)
        mtab[:, 0] = kbias[g, 128 * c : 128 * (c + 1)]
        mtab[:, 1] = kbias[g, 128 * (7 - c) : 128 * (8 - c)]
        for j in range(3):
            mtab[:, 2 + j] = (kbias[g, 128 * j : 128 * (j + 1)] if j < c else NEG)
        for j in range(7):
            mtab[:, 5 + j] = (kbias[g, 128 * j : 128 * (j + 1)] if j <= 6 - c else NEG)
        lmT = np.ascontiguousarray(lm_pad[VS * cid : VS * (cid + 1)].T).astype(BF16_NP)
        in_maps.append({
            "x0t": x0t, "tri": tri, "mtab": mtab,
            "wqT": wqT, "wkT": wkT, "wvT": wvT, "woT": woT, "w1T": w1T, "w2T": w2T,
            "lmT": lmT,
        })

    if n_layers not in _prog_cache:
        _prog_cache[n_layers] = _build_program(n_layers)
    nc = _prog_cache[n_layers]
    import os
    _trace = bool(os.environ.get("KTRACE"))
    res = None
    for attempt in range(3):
        try:
            res = run_bass_kernel_spmd(nc, in_maps, core_ids=list(range(NC_)),
                                       trace=_trace and attempt == 0)
            break
        except ModuleNotFoundError:
            _trace = False
        except Exception:
            if attempt == 2:
                raise
    if res is None:
        res = run_bass_kernel_spmd(nc, in_maps, core_ids=list(range(NC_)))
    global last_exec_time_ns
    last_exec_time_ns = res.exec_time_ns
    full = np.concatenate([res.results[c]["logits_sh"] for c in range(NC_)], axis=1)
    return np.ascontiguousarray(full[:, :V].reshape(B, S, V).astype(np.float32))
